# revision 26
# baseline (speedup 1.0000x reference)
"""LoRA attention kernel for 8 Trainium2 NeuronCores.

Sharding: core = (b, qhalf, ghalf).  Each core handles batch b, one half of
the query positions (1024 of 2048), and one half of the heads (8 of 16).
K/V are projected for the unmasked tokens only (keys with mask=0 contribute
nothing to masked softmax, and the mask is known host-side): tokens are
compacted to NKC=1152 padded columns (~1024 real), cutting K/V projection,
QK^T, exp, and P@V work by ~37%.  Q is projected for the core's 1024
queries.  LoRA paths and the score scale fold host-side into bf16 weights.

Scores are computed transposed (ST[m, q]); the softmax denominator comes
from an augmented v column (v rows scaled by the compacted mask, aug col =
mask).  All matmuls are bf16 with fp32 PSUM accumulation; exp runs on
ScalarE over [128, 1024] 2-bank PSUM tiles.  The attention t-loop is
software-pipelined (QK(t+1) issued before PV(t)) and the K/Q projections
for later head-pairs are emitted between head loops so ScalarE starts
early.  Per-head outputs are exchanged via two 2-rank AllGathers (heads
0-3 early, overlapping compute), and each core computes a 512-row slice of
the output projection.
"""

import sys
from contextlib import ExitStack

import numpy as np

for _p in ("/opt/trn_rl_repo", "/opt/trn_rl_repo/concourse"):
    if _p not in sys.path:
        sys.path.insert(0, _p)

import concourse.bass as bass
import concourse.mybir as mybir
import concourse.tile as tile
from concourse import bacc
from concourse import bass_utils

import ml_dtypes

BF16 = mybir.dt.bfloat16
F32 = mybir.dt.float32
EXP = mybir.ActivationFunctionType.Exp
NPBF16 = ml_dtypes.bfloat16

H, D, DIM, R = 16, 64, 1024, 10
B, N = 2, 2048
NCORES = 8
ATT = float(D) ** -0.5
LS = 1.0 / R

HPC = 8               # heads per core
HD = HPC * D          # 512 qkv rows per core per projection
NQ = N // 2           # 1024 queries per core
NKC = 1152            # compacted+padded key tokens (~1024 unmasked + pad)
KT = DIM // 128       # 8 contraction tiles
NTK = NKC // 128      # 10 key-token tiles
HT = HPC // 2         # 4 head-pair tiles (128 rows each)
CT = 512 // 128       # 4 output row tiles per core
KCH = (512, 512, 128)  # key-token chunking for the K projection
GROUPS = [[0, 1], [2, 3], [4, 5], [6, 7]]

# test harness hooks
TRACE = False
TRACE_DIR = None
LAST_RESULTS = None

_NC_CACHE = None


def _build_nc():
    nc = bacc.Bacc(None, target_bir_lowering=False, num_devices=NCORES)

    xqT = nc.dram_tensor("xqT", (DIM, NQ), BF16, kind="ExternalInput")
    xkT = nc.dram_tensor("xkT", (DIM, NKC), BF16, kind="ExternalInput")
    wqT = nc.dram_tensor("wqT", (DIM, HD), BF16, kind="ExternalInput")
    wkT = nc.dram_tensor("wkT", (DIM, HD), BF16, kind="ExternalInput")
    wvT = nc.dram_tensor("wvT", (DIM, HD), BF16, kind="ExternalInput")
    bq = nc.dram_tensor("bq", (HD,), F32, kind="ExternalInput")
    bv = nc.dram_tensor("bv", (1, HD), BF16, kind="ExternalInput")
    mk = nc.dram_tensor("mk", (NKC,), F32, kind="ExternalInput")
    woT = nc.dram_tensor("woT", (DIM, HD), BF16, kind="ExternalInput")
    bo = nc.dram_tensor("bo", (HD,), F32, kind="ExternalInput")
    outT = nc.dram_tensor("outT", (HD, NQ), F32, kind="ExternalOutput")

    agins = [nc.dram_tensor(f"agin{i}", (128, NQ), BF16) for i in range(4)]
    agouts = [nc.dram_tensor(f"agout{i}", (256, NQ), BF16) for i in range(4)]
    recd = nc.dram_tensor("recd", (HPC, NQ), F32)

    with ExitStack() as ctx:
        tc = ctx.enter_context(tile.TileContext(nc))
        const = ctx.enter_context(tc.tile_pool(name="const", bufs=1))

        mk_sb = const.tile([128, NTK], F32)
        bq_sb = const.tile([128, HT], F32)
        bo_sb = const.tile([128, CT], F32)
        bv_sb = const.tile([1, HD], BF16)
        ones_sb = const.tile([1, 128], BF16)
        nc.vector.memset(ones_sb, 1.0)
        ones8 = const.tile([128, HPC], F32)
        nc.vector.memset(ones8, 1.0)
        woT_sb = const.tile([128, KT, HD], BF16)

        kT_sb = const.tile([128, HT, NKC], BF16)   # [(h%2)*64+d, hpair, m]
        qT_sb = const.tile([128, HT, NQ], BF16)    # [(h%2)*64+d, hpair, q]
        vsb = const.tile([128, NTK, HPC, D + 1], BF16)  # v rows + mask col

        xw = ctx.enter_context(tc.tile_pool(name="xw", bufs=1))
        xkT_sb = xw.tile([128, KT, NKC], BF16)
        xqT_sb = xw.tile([128, KT, NQ], BF16)
        wqT_sb = xw.tile([128, KT, HD], BF16)
        wkT_sb = xw.tile([128, KT, HD], BF16)
        wvT_sb = xw.tile([128, KT, HD], BF16)
        # chunked input DMAs, round-robin across the 3 DMA-capable
        # engines, ordered by first use (K proj -> Q proj -> V proj)
        wkT_r = wkT[:, :].rearrange("(k p) m -> p k m", p=128)
        wqT_r = wqT[:, :].rearrange("(k p) m -> p k m", p=128)
        wvT_r = wvT[:, :].rearrange("(k p) m -> p k m", p=128)
        xkT_r = xkT[:, :].rearrange("(k p) n -> p k n", p=128)
        xqT_r = xqT[:, :].rearrange("(k p) n -> p k n", p=128)
        # K-projection inputs (wk, xk) interleaved on the two fast HWDGE
        # queues first; Q inputs next; V weights + small consts last.
        for k in range(KT):
            nc.sync.dma_start(out=wkT_sb[:, k, :], in_=wkT_r[:, k, :])
            nc.scalar.dma_start(out=xkT_sb[:, k, :], in_=xkT_r[:, k, :])
        for k in range(KT):
            nc.sync.dma_start(out=xqT_sb[:, k, :], in_=xqT_r[:, k, :])
            nc.scalar.dma_start(out=wqT_sb[:, k, :], in_=wqT_r[:, k, :])
            nc.gpsimd.dma_start(out=wvT_sb[:, k, :], in_=wvT_r[:, k, :])
        nc.gpsimd.dma_start(out=mk_sb,
                            in_=mk[:].rearrange("(t p) -> p t", p=128))
        nc.gpsimd.dma_start(out=bq_sb,
                            in_=bq[:].rearrange("(i p) -> p i", p=128))
        nc.gpsimd.dma_start(out=bo_sb,
                            in_=bo[:].rearrange("(c p) -> p c", p=128))
        nc.gpsimd.dma_start(out=bv_sb, in_=bv[:, :])

        agp = ctx.enter_context(tc.tile_pool(name="agp", bufs=1))
        agT = agp.tile([128, KT, NQ], BF16)
        ag_rs = [a[:, :].rearrange("(k p) n -> p k n", p=128) for a in agouts]
        ppo = ctx.enter_context(
            tc.tile_pool(name="pp_o", bufs=2, space="PSUM"))

        ictx = ctx.enter_context(ExitStack())
        ppkq = ictx.enter_context(
            tc.tile_pool(name="pp_kq", bufs=1, space="PSUM"))

        def proj_v():
            # V untransposed: V[m, hd] = x[m, :] @ WvT + bv, masked, + aug col
            for t in range(NTK):
                pv = ppkq.tile([128, 512], F32, tag="pk", name=f"pv{t}")
                for k in range(KT):
                    nc.tensor.matmul(
                        pv, lhsT=xkT_sb[:, k, t * 128:(t + 1) * 128],
                        rhs=wvT_sb[:, k, :],
                        start=(k == 0), stop=False,
                    )
                nc.tensor.matmul(pv, lhsT=ones_sb, rhs=bv_sb,
                                 start=False, stop=True)
                nc.vector.tensor_scalar_mul(
                    vsb[:, t, :, 0:D],
                    pv[:, :].rearrange("p (h d) -> p h d", h=HPC),
                    mk_sb[:, t:t + 1],
                )
                nc.vector.tensor_scalar_mul(
                    vsb[:, t, :, D:D + 1], ones8[:, :], mk_sb[:, t:t + 1])

        def proj_kq(i):
            # K rows for head-pair i over compacted keys
            coff = 0
            for csz in KCH:
                ps = ppkq.tile([128, 512], F32, tag="pk", name=f"pk{i}_{coff}")
                for k in range(KT):
                    nc.tensor.matmul(
                        ps[:, 0:csz], lhsT=wkT_sb[:, k, i * 128:(i + 1) * 128],
                        rhs=xkT_sb[:, k, coff:coff + csz],
                        start=(k == 0), stop=(k == KT - 1),
                    )
                nc.vector.tensor_copy(
                    kT_sb[:, i, coff:coff + csz], ps[:, 0:csz])
                coff += csz
            # Q rows for head-pair i over this core's queries
            for j in range(2):
                ps = ppkq.tile([128, 512], F32, tag="pk", name=f"pq{i}_{j}")
                for k in range(KT):
                    nc.tensor.matmul(
                        ps, lhsT=wqT_sb[:, k, i * 128:(i + 1) * 128],
                        rhs=xqT_sb[:, k, j * 512:(j + 1) * 512],
                        start=(k == 0), stop=(k == KT - 1),
                    )
                nc.vector.tensor_scalar_add(
                    qT_sb[:, i, j * 512:(j + 1) * 512], ps, bq_sb[:, i:i + 1])

        proj_kq(0)
        nc.sync.dma_start(
            out=woT_sb,
            in_=woT[:, :].rearrange("(k p) c -> p k c", p=128))
        proj_v()

        # ---- attention, software-pipelined; K/Q proj for head-pair i+1
        # emitted between head pairs ----
        expool = ictx.enter_context(tc.tile_pool(name="expool", bufs=4))
        attp = ictx.enter_context(tc.tile_pool(name="attp", bufs=2))
        recbp = ictx.enter_context(tc.tile_pool(name="recbp", bufs=2))
        ppst = ictx.enter_context(
            tc.tile_pool(name="pp_st", bufs=3, space="PSUM"))

        def qk1(h, t, j):
            ih, off = h // 2, (h % 2) * 64
            st = ppst.tile([128, 512], F32, tag="st", name=f"st{h}_{t}_{j}")
            nc.tensor.matmul(
                st, lhsT=kT_sb[off:off + 64, ih, t * 128:(t + 1) * 128],
                rhs=qT_sb[off:off + 64, ih, j * 512:(j + 1) * 512],
                start=True, stop=True,
            )
            return st

        for h in range(HPC):
            op = ppo.tile([128, NQ], F32, tag="op")
            cur = [qk1(h, 0, 0), qk1(h, 0, 1)]
            for t in range(NTK):
                ex = expool.tile([128, NQ], BF16, tag="ex", name=f"ex{h}_{t}")
                nxt = [None, None]
                if t + 1 < NTK:
                    nxt[0] = qk1(h, t + 1, 0)
                nc.scalar.activation(ex[:, 0:512], cur[0], EXP)
                if t + 1 < NTK:
                    nxt[1] = qk1(h, t + 1, 1)
                nc.scalar.activation(ex[:, 512:1024], cur[1], EXP)
                vlhs = vsb[:, t, h, :]
                for j in range(2):
                    nc.tensor.matmul(
                        op[0:D + 1, j * 512:(j + 1) * 512],
                        lhsT=vlhs, rhs=ex[:, j * 512:(j + 1) * 512],
                        start=(t == 0), stop=(t == NTK - 1),
                    )
                cur = nxt
            # normalize rows 0..63 by the denominator row 64
            # (reciprocal_approx_fast misreads PSUM on HW: copy to SBUF first)
            den_s = recbp.tile([1, NQ], F32, tag="den_s")
            nc.vector.tensor_copy(den_s, op[D:D + 1, :])
            rec_s = recbp.tile([1, NQ], F32, tag="rec_s")
            nc.vector.reciprocal_approx_fast(rec_s, den_s)
            nc.sync.dma_start(out=recd[h:h + 1, :], in_=rec_s)
            rec = recbp.tile([64, NQ], F32, tag="rec")
            rsrc = recd[h:h + 1, :]
            nc.sync.dma_start(
                out=rec,
                in_=bass.AP(tensor=rsrc.tensor, offset=rsrc.offset,
                            ap=[[0, 64], [1, NQ]]),
            )
            att = attp.tile([64, NQ], BF16, tag="att")
            nc.vector.tensor_mul(att, op[0:D, :], rec)
            nc.sync.dma_start(
                out=agins[h // 2][(h % 2) * 64:(h % 2 + 1) * 64, :], in_=att)
            if h % 2 == 1:
                nc.gpsimd.collective_compute(
                    "AllGather", mybir.AluOpType.bypass,
                    replica_groups=GROUPS,
                    ins=[agins[h // 2][:, :].opt()],
                    outs=[agouts[h // 2][:, :].opt()],
                )
            if h < HPC - 1 and h % 2 == 0 and h // 2 + 1 < HT:
                proj_kq(h // 2 + 1)
        # agT loads emitted after every AG issue so no blocked DMA delays a
        # collective trigger; only the k6/k7 loads wait on the last AG
        for a in range(4):
            for k in range(2):
                nc.gpsimd.dma_start(out=agT[:, a * 2 + k, :],
                                    in_=ag_rs[a][:, k, :])

        # ---- output projection slice ----
        # c0/c1 share the attention op slots (k0-3 accumulate during late
        # attention, agout0 data is ready); c2/c3 use banks freed when the
        # attention pools close (run during the AG1 window).
        def fproj_start(c, fp):
            for k in range(6):
                lhs = woT_sb[:, k, c * 128:(c + 1) * 128]
                for j in range(2):
                    nc.tensor.matmul(
                        fp[:, j * 512:(j + 1) * 512], lhsT=lhs,
                        rhs=agT[:, k, j * 512:(j + 1) * 512],
                        start=(k == 0), stop=False,
                    )

        fps = []
        for c in range(2):
            fp = ppo.tile([128, NQ], F32, tag="op", name=f"fp{c}")
            fproj_start(c, fp)
            fps.append(fp)
        ictx.close()

        with tc.tile_pool(name="outp", bufs=2) as outp, \
             tc.tile_pool(name="pp_f", bufs=2, space="PSUM") as ppf:
            for c in range(2, CT):
                fp = ppf.tile([128, NQ], F32, tag="fp", name=f"fp{c}")
                fproj_start(c, fp)
                fps.append(fp)
            out_r = outT[:, :].rearrange("(c p) n -> p c n", p=128)
            for c in range(CT):
                fp = fps[c]
                for k in range(6, KT):
                    lhs = woT_sb[:, k, c * 128:(c + 1) * 128]
                    for j in range(2):
                        nc.tensor.matmul(
                            fp[:, j * 512:(j + 1) * 512], lhsT=lhs,
                            rhs=agT[:, k, j * 512:(j + 1) * 512],
                            start=False, stop=(k == KT - 1),
                        )
                ot = outp.tile([128, NQ], F32, tag="ot")
                nc.vector.tensor_scalar_add(ot, fp, bo_sb[:, c:c + 1])
                nc.sync.dma_start(out=out_r[:, c, :], in_=ot)

    nc.finalize()
    return nc


def _bf16(a):
    return np.ascontiguousarray(np.asarray(a, np.float32).astype(NPBF16))


def _prep_core_inputs(inputs, c):
    b, qh, g = c // 4, (c // 2) % 2, c % 2
    rows = slice(g * HD, (g + 1) * HD)
    w_qkv = np.asarray(inputs["w_qkv"], np.float32)
    Wq = (w_qkv[0:H * D][rows]
          + np.asarray(inputs["wq_base"], np.float32)[rows]
          + LS * (np.asarray(inputs["wq_B"], np.float32)[rows]
                  @ np.asarray(inputs["wq_A"], np.float32))) * ATT
    Wk = w_qkv[H * D:2 * H * D][rows]
    Wv = (w_qkv[2 * H * D:3 * H * D][rows]
          + np.asarray(inputs["wv_base"], np.float32)[rows]
          + LS * (np.asarray(inputs["wv_B"], np.float32)[rows]
                  @ np.asarray(inputs["wv_A"], np.float32)))
    bqv = (np.asarray(inputs["bq_base"], np.float32)[rows] * ATT)
    bvv = np.asarray(inputs["bv_base"], np.float32)[rows]

    x = np.asarray(inputs["x"], np.float32)[b]          # (N, DIM)
    mask = np.asarray(inputs["mask"]).astype(bool)[b]
    xq = np.roll(x, -qh * NQ, axis=0)[0:NQ]             # this core's queries

    # compact keys to unmasked tokens, pad to NKC
    idx = np.nonzero(mask)[0]
    cnt = min(len(idx), NKC)
    xk = np.zeros((NKC, DIM), np.float32)
    xk[:cnt] = x[idx[:cnt]]
    mkc = np.zeros(NKC, np.float32)
    mkc[:cnt] = 1.0

    # final projection contraction order must match agout row order:
    # agout[a] = [g0 heads 2a,2a+1 | g1 heads 2a,2a+1]
    perm = np.concatenate([
        np.concatenate([np.arange(a * 128, (a + 1) * 128),
                        np.arange(512 + a * 128, 512 + (a + 1) * 128)])
        for a in range(4)])
    w_out = np.asarray(inputs["w_out"], np.float32)
    orows = slice(g * HD, (g + 1) * HD)
    woTv = w_out[orows][:, perm].T                      # (DIM, 512)
    bov = np.asarray(inputs["b_out"], np.float32)[orows]

    return {
        "xqT": _bf16(xq.T), "xkT": _bf16(xk.T),
        "wqT": _bf16(Wq.T), "wkT": _bf16(Wk.T), "wvT": _bf16(Wv.T),
        "bq": np.ascontiguousarray(bqv), "bv": _bf16(bvv[None, :]),
        "mk": np.ascontiguousarray(mkc),
        "woT": _bf16(woTv), "bo": np.ascontiguousarray(bov),
    }


def kernel(**inputs):
    global _NC_CACHE, LAST_RESULTS
    if _NC_CACHE is None:
        _NC_CACHE = _build_nc()
    nc = _NC_CACHE
    in_maps = [_prep_core_inputs(inputs, c) for c in range(NCORES)]
    res = bass_utils.run_bass_kernel_spmd(
        nc, in_maps, core_ids=list(range(NCORES)),
        trace=TRACE, tmpdir=TRACE_DIR,
    )
    LAST_RESULTS = res
    out = np.empty((B, N, DIM), np.float32)
    for c in range(NCORES):
        b, qh, g = c // 4, (c // 2) % 2, c % 2
        out[b, qh * NQ:(qh + 1) * NQ, g * HD:(g + 1) * HD] = \
            res.results[c]["outT"].T
    return out


# revision 27
# speedup vs baseline: 1.0299x; 1.0299x over previous
"""LoRA attention kernel for 8 Trainium2 NeuronCores.

Sharding: core = (b, qhalf, ghalf).  Each core handles batch b, one half of
the query positions (1024 of 2048), and one half of the heads (8 of 16).
K/V are projected for the unmasked tokens only (keys with mask=0 contribute
nothing to masked softmax, and the mask is known host-side): tokens are
compacted to NKC=1152 padded columns (~1024 real), cutting K/V projection,
QK^T, exp, and P@V work by ~37%.  Q is projected for the core's 1024
queries.  LoRA paths and the score scale fold host-side into bf16 weights.

Scores are computed transposed (ST[m, q]); the softmax denominator comes
from an augmented v column (v rows scaled by the compacted mask, aug col =
mask).  All matmuls are bf16 with fp32 PSUM accumulation; exp runs on
ScalarE over [128, 1024] 2-bank PSUM tiles.  The attention t-loop is
software-pipelined (QK(t+1) issued before PV(t)) and the K/Q projections
for later head-pairs are emitted between head loops so ScalarE starts
early.  Per-head outputs are exchanged via two 2-rank AllGathers (heads
0-3 early, overlapping compute), and each core computes a 512-row slice of
the output projection.
"""

import sys
from contextlib import ExitStack

import numpy as np

for _p in ("/opt/trn_rl_repo", "/opt/trn_rl_repo/concourse"):
    if _p not in sys.path:
        sys.path.insert(0, _p)

import concourse.bass as bass
import concourse.mybir as mybir
import concourse.tile as tile
from concourse import bacc
from concourse import bass_utils

import ml_dtypes

BF16 = mybir.dt.bfloat16
F32 = mybir.dt.float32
EXP = mybir.ActivationFunctionType.Exp
NPBF16 = ml_dtypes.bfloat16

H, D, DIM, R = 16, 64, 1024, 10
B, N = 2, 2048
NCORES = 8
ATT = float(D) ** -0.5
LS = 1.0 / R

HPC = 8               # heads per core
HD = HPC * D          # 512 qkv rows per core per projection
NQ = N // 2           # 1024 queries per core
NKC = 1152            # compacted+padded key tokens (~1024 unmasked + pad)
KT = DIM // 128       # 8 contraction tiles
NTK = NKC // 128      # 10 key-token tiles
HT = HPC // 2         # 4 head-pair tiles (128 rows each)
CT = 512 // 128       # 4 output row tiles per core
KCH = (512, 512, 128)  # key-token chunking for the K projection
GROUPS = [[0, 1], [2, 3], [4, 5], [6, 7]]

# test harness hooks
TRACE = False
TRACE_DIR = None
LAST_RESULTS = None

_NC_CACHE = None


def _build_nc():
    nc = bacc.Bacc(None, target_bir_lowering=False, num_devices=NCORES)

    xqT = nc.dram_tensor("xqT", (DIM, NQ), BF16, kind="ExternalInput")
    xkT = nc.dram_tensor("xkT", (DIM, NKC), BF16, kind="ExternalInput")
    wqT = nc.dram_tensor("wqT", (DIM, HD), BF16, kind="ExternalInput")
    wkT = nc.dram_tensor("wkT", (DIM, HD), BF16, kind="ExternalInput")
    wvT = nc.dram_tensor("wvT", (DIM, HD), BF16, kind="ExternalInput")
    bq = nc.dram_tensor("bq", (HD,), F32, kind="ExternalInput")
    bv = nc.dram_tensor("bv", (1, HD), BF16, kind="ExternalInput")
    mk = nc.dram_tensor("mk", (NKC,), F32, kind="ExternalInput")
    woT = nc.dram_tensor("woT", (DIM, HD), BF16, kind="ExternalInput")
    bo = nc.dram_tensor("bo", (HD,), F32, kind="ExternalInput")
    outT = nc.dram_tensor("outT", (HD, NQ), F32, kind="ExternalOutput")

    agins = [nc.dram_tensor(f"agin{i}", (128, NQ), BF16) for i in range(4)]
    agouts = [nc.dram_tensor(f"agout{i}", (256, NQ), BF16) for i in range(4)]
    recd = nc.dram_tensor("recd", (HPC, NQ), F32)

    with ExitStack() as ctx:
        tc = ctx.enter_context(tile.TileContext(nc))
        const = ctx.enter_context(tc.tile_pool(name="const", bufs=1))

        mk_sb = const.tile([128, NTK], F32)
        nc.gpsimd.dma_start(out=mk_sb,
                            in_=mk[:].rearrange("(t p) -> p t", p=128))
        bq_sb = const.tile([128, HT], F32)
        nc.gpsimd.dma_start(out=bq_sb,
                            in_=bq[:].rearrange("(i p) -> p i", p=128))
        bo_sb = const.tile([128, CT], F32)
        nc.gpsimd.dma_start(out=bo_sb,
                            in_=bo[:].rearrange("(c p) -> p c", p=128))
        bv_sb = const.tile([1, HD], BF16)
        nc.gpsimd.dma_start(out=bv_sb, in_=bv[:, :])
        ones_sb = const.tile([1, 128], BF16)
        nc.vector.memset(ones_sb, 1.0)
        ones8 = const.tile([128, HPC], F32)
        nc.vector.memset(ones8, 1.0)
        woT_sb = const.tile([128, KT, HD], BF16)

        kT_sb = const.tile([128, HT, NKC], BF16)   # [(h%2)*64+d, hpair, m]
        qT_sb = const.tile([128, HT, NQ], BF16)    # [(h%2)*64+d, hpair, q]
        vsb = const.tile([128, NTK, HPC, D + 1], BF16)  # v rows + mask col

        xw = ctx.enter_context(tc.tile_pool(name="xw", bufs=1))
        xkT_sb = xw.tile([128, KT, NKC], BF16)
        xqT_sb = xw.tile([128, KT, NQ], BF16)
        wqT_sb = xw.tile([128, KT, HD], BF16)
        wkT_sb = xw.tile([128, KT, HD], BF16)
        wvT_sb = xw.tile([128, KT, HD], BF16)
        # chunked input DMAs, round-robin across the 3 DMA-capable
        # engines, ordered by first use (K proj -> Q proj -> V proj)
        wkT_r = wkT[:, :].rearrange("(k p) m -> p k m", p=128)
        wqT_r = wqT[:, :].rearrange("(k p) m -> p k m", p=128)
        wvT_r = wvT[:, :].rearrange("(k p) m -> p k m", p=128)
        xkT_r = xkT[:, :].rearrange("(k p) n -> p k n", p=128)
        xqT_r = xqT[:, :].rearrange("(k p) n -> p k n", p=128)
        engs = (nc.sync, nc.scalar, nc.gpsimd)
        qi = 0
        for dst, srcr in ((wkT_sb, wkT_r), (xkT_sb, xkT_r),
                          (wqT_sb, wqT_r), (xqT_sb, xqT_r),
                          (wvT_sb, wvT_r)):
            for k in range(KT):
                engs[qi % 3].dma_start(out=dst[:, k, :], in_=srcr[:, k, :])
                qi += 1

        agp = ctx.enter_context(tc.tile_pool(name="agp", bufs=1))
        agT = agp.tile([128, KT, NQ], BF16)
        ag_rs = [a[:, :].rearrange("(k p) n -> p k n", p=128) for a in agouts]
        ppo = ctx.enter_context(
            tc.tile_pool(name="pp_o", bufs=2, space="PSUM"))

        ictx = ctx.enter_context(ExitStack())
        ppkq = ictx.enter_context(
            tc.tile_pool(name="pp_kq", bufs=1, space="PSUM"))

        def proj_v():
            # V untransposed: V[m, hd] = x[m, :] @ WvT + bv, masked, + aug col
            for t in range(NTK):
                pv = ppkq.tile([128, 512], F32, tag="pk", name=f"pv{t}")
                for k in range(KT):
                    nc.tensor.matmul(
                        pv, lhsT=xkT_sb[:, k, t * 128:(t + 1) * 128],
                        rhs=wvT_sb[:, k, :],
                        start=(k == 0), stop=False,
                    )
                nc.tensor.matmul(pv, lhsT=ones_sb, rhs=bv_sb,
                                 start=False, stop=True)
                nc.vector.tensor_scalar_mul(
                    vsb[:, t, :, 0:D],
                    pv[:, :].rearrange("p (h d) -> p h d", h=HPC),
                    mk_sb[:, t:t + 1],
                )
                nc.vector.tensor_scalar_mul(
                    vsb[:, t, :, D:D + 1], ones8[:, :], mk_sb[:, t:t + 1])

        def proj_kq(i):
            # K rows for head-pair i over compacted keys
            coff = 0
            for csz in KCH:
                ps = ppkq.tile([128, 512], F32, tag="pk", name=f"pk{i}_{coff}")
                for k in range(KT):
                    nc.tensor.matmul(
                        ps[:, 0:csz], lhsT=wkT_sb[:, k, i * 128:(i + 1) * 128],
                        rhs=xkT_sb[:, k, coff:coff + csz],
                        start=(k == 0), stop=(k == KT - 1),
                    )
                nc.vector.tensor_copy(
                    kT_sb[:, i, coff:coff + csz], ps[:, 0:csz])
                coff += csz
            # Q rows for head-pair i over this core's queries
            for j in range(2):
                ps = ppkq.tile([128, 512], F32, tag="pk", name=f"pq{i}_{j}")
                for k in range(KT):
                    nc.tensor.matmul(
                        ps, lhsT=wqT_sb[:, k, i * 128:(i + 1) * 128],
                        rhs=xqT_sb[:, k, j * 512:(j + 1) * 512],
                        start=(k == 0), stop=(k == KT - 1),
                    )
                nc.vector.tensor_scalar_add(
                    qT_sb[:, i, j * 512:(j + 1) * 512], ps, bq_sb[:, i:i + 1])

        proj_kq(0)
        nc.sync.dma_start(
            out=woT_sb,
            in_=woT[:, :].rearrange("(k p) c -> p k c", p=128))
        proj_v()

        # ---- attention, software-pipelined; K/Q proj for head-pair i+1
        # emitted between head pairs ----
        expool = ictx.enter_context(tc.tile_pool(name="expool", bufs=4))
        attp = ictx.enter_context(tc.tile_pool(name="attp", bufs=2))
        recbp = ictx.enter_context(tc.tile_pool(name="recbp", bufs=2))
        ppst = ictx.enter_context(
            tc.tile_pool(name="pp_st", bufs=3, space="PSUM"))

        def qk1(h, t, j):
            ih, off = h // 2, (h % 2) * 64
            st = ppst.tile([128, 512], F32, tag="st", name=f"st{h}_{t}_{j}")
            nc.tensor.matmul(
                st, lhsT=kT_sb[off:off + 64, ih, t * 128:(t + 1) * 128],
                rhs=qT_sb[off:off + 64, ih, j * 512:(j + 1) * 512],
                start=True, stop=True,
            )
            return st

        for h in range(HPC):
            op = ppo.tile([128, NQ], F32, tag="op")
            cur = [qk1(h, 0, 0), qk1(h, 0, 1)]
            for t in range(NTK):
                ex = expool.tile([128, NQ], BF16, tag="ex", name=f"ex{h}_{t}")
                nxt = [None, None]
                if t + 1 < NTK:
                    nxt[0] = qk1(h, t + 1, 0)
                nc.scalar.activation(ex[:, 0:512], cur[0], EXP)
                if t + 1 < NTK:
                    nxt[1] = qk1(h, t + 1, 1)
                nc.scalar.activation(ex[:, 512:1024], cur[1], EXP)
                vlhs = vsb[:, t, h, :]
                for j in range(2):
                    nc.tensor.matmul(
                        op[0:D + 1, j * 512:(j + 1) * 512],
                        lhsT=vlhs, rhs=ex[:, j * 512:(j + 1) * 512],
                        start=(t == 0), stop=(t == NTK - 1),
                    )
                cur = nxt
            # normalize rows 0..63 by the denominator row 64
            # (reciprocal_approx_fast misreads PSUM on HW: copy to SBUF first)
            den_s = recbp.tile([1, NQ], F32, tag="den_s")
            nc.vector.tensor_copy(den_s, op[D:D + 1, :])
            rec_s = recbp.tile([1, NQ], F32, tag="rec_s")
            nc.vector.reciprocal_approx_fast(rec_s, den_s)
            nc.sync.dma_start(out=recd[h:h + 1, :], in_=rec_s)
            rec = recbp.tile([64, NQ], F32, tag="rec")
            rsrc = recd[h:h + 1, :]
            nc.sync.dma_start(
                out=rec,
                in_=bass.AP(tensor=rsrc.tensor, offset=rsrc.offset,
                            ap=[[0, 64], [1, NQ]]),
            )
            att = attp.tile([64, NQ], BF16, tag="att")
            nc.vector.tensor_mul(att, op[0:D, :], rec)
            nc.sync.dma_start(
                out=agins[h // 2][(h % 2) * 64:(h % 2 + 1) * 64, :], in_=att)
            if h % 2 == 1:
                nc.gpsimd.collective_compute(
                    "AllGather", mybir.AluOpType.bypass,
                    replica_groups=GROUPS,
                    ins=[agins[h // 2][:, :].opt()],
                    outs=[agouts[h // 2][:, :].opt()],
                )
            if h < HPC - 1 and h % 2 == 0 and h // 2 + 1 < HT:
                proj_kq(h // 2 + 1)
        # agT loads emitted after every AG issue so no blocked DMA delays a
        # collective trigger; only the k6/k7 loads wait on the last AG
        for a in range(4):
            for k in range(2):
                nc.gpsimd.dma_start(out=agT[:, a * 2 + k, :],
                                    in_=ag_rs[a][:, k, :])

        # ---- output projection slice ----
        # c0/c1 share the attention op slots (k0-3 accumulate during late
        # attention, agout0 data is ready); c2/c3 use banks freed when the
        # attention pools close (run during the AG1 window).
        def fproj_start(c, fp):
            for k in range(6):
                lhs = woT_sb[:, k, c * 128:(c + 1) * 128]
                for j in range(2):
                    nc.tensor.matmul(
                        fp[:, j * 512:(j + 1) * 512], lhsT=lhs,
                        rhs=agT[:, k, j * 512:(j + 1) * 512],
                        start=(k == 0), stop=False,
                    )

        fps = []
        for c in range(2):
            fp = ppo.tile([128, NQ], F32, tag="op", name=f"fp{c}")
            fproj_start(c, fp)
            fps.append(fp)
        ictx.close()

        with tc.tile_pool(name="outp", bufs=2) as outp, \
             tc.tile_pool(name="pp_f", bufs=2, space="PSUM") as ppf:
            for c in range(2, CT):
                fp = ppf.tile([128, NQ], F32, tag="fp", name=f"fp{c}")
                fproj_start(c, fp)
                fps.append(fp)
            out_r = outT[:, :].rearrange("(c p) n -> p c n", p=128)
            for c in range(CT):
                fp = fps[c]
                for k in range(6, KT):
                    lhs = woT_sb[:, k, c * 128:(c + 1) * 128]
                    for j in range(2):
                        nc.tensor.matmul(
                            fp[:, j * 512:(j + 1) * 512], lhsT=lhs,
                            rhs=agT[:, k, j * 512:(j + 1) * 512],
                            start=False, stop=(k == KT - 1),
                        )
                ot = outp.tile([128, NQ], F32, tag="ot")
                nc.vector.tensor_scalar_add(ot, fp, bo_sb[:, c:c + 1])
                nc.sync.dma_start(out=out_r[:, c, :], in_=ot)

    nc.finalize()
    return nc


def _bf16(a):
    return np.ascontiguousarray(np.asarray(a, np.float32).astype(NPBF16))


def _prep_core_inputs(inputs, c):
    b, qh, g = c // 4, (c // 2) % 2, c % 2
    rows = slice(g * HD, (g + 1) * HD)
    w_qkv = np.asarray(inputs["w_qkv"], np.float32)
    Wq = (w_qkv[0:H * D][rows]
          + np.asarray(inputs["wq_base"], np.float32)[rows]
          + LS * (np.asarray(inputs["wq_B"], np.float32)[rows]
                  @ np.asarray(inputs["wq_A"], np.float32))) * ATT
    Wk = w_qkv[H * D:2 * H * D][rows]
    Wv = (w_qkv[2 * H * D:3 * H * D][rows]
          + np.asarray(inputs["wv_base"], np.float32)[rows]
          + LS * (np.asarray(inputs["wv_B"], np.float32)[rows]
                  @ np.asarray(inputs["wv_A"], np.float32)))
    bqv = (np.asarray(inputs["bq_base"], np.float32)[rows] * ATT)
    bvv = np.asarray(inputs["bv_base"], np.float32)[rows]

    x = np.asarray(inputs["x"], np.float32)[b]          # (N, DIM)
    mask = np.asarray(inputs["mask"]).astype(bool)[b]
    xq = np.roll(x, -qh * NQ, axis=0)[0:NQ]             # this core's queries

    # compact keys to unmasked tokens, pad to NKC
    idx = np.nonzero(mask)[0]
    cnt = min(len(idx), NKC)
    xk = np.zeros((NKC, DIM), np.float32)
    xk[:cnt] = x[idx[:cnt]]
    mkc = np.zeros(NKC, np.float32)
    mkc[:cnt] = 1.0

    # final projection contraction order must match agout row order:
    # agout[a] = [g0 heads 2a,2a+1 | g1 heads 2a,2a+1]
    perm = np.concatenate([
        np.concatenate([np.arange(a * 128, (a + 1) * 128),
                        np.arange(512 + a * 128, 512 + (a + 1) * 128)])
        for a in range(4)])
    w_out = np.asarray(inputs["w_out"], np.float32)
    orows = slice(g * HD, (g + 1) * HD)
    woTv = w_out[orows][:, perm].T                      # (DIM, 512)
    bov = np.asarray(inputs["b_out"], np.float32)[orows]

    return {
        "xqT": _bf16(xq.T), "xkT": _bf16(xk.T),
        "wqT": _bf16(Wq.T), "wkT": _bf16(Wk.T), "wvT": _bf16(Wv.T),
        "bq": np.ascontiguousarray(bqv), "bv": _bf16(bvv[None, :]),
        "mk": np.ascontiguousarray(mkc),
        "woT": _bf16(woTv), "bo": np.ascontiguousarray(bov),
    }


def kernel(**inputs):
    global _NC_CACHE, LAST_RESULTS
    if _NC_CACHE is None:
        _NC_CACHE = _build_nc()
    nc = _NC_CACHE
    in_maps = [_prep_core_inputs(inputs, c) for c in range(NCORES)]
    res = bass_utils.run_bass_kernel_spmd(
        nc, in_maps, core_ids=list(range(NCORES)),
        trace=TRACE, tmpdir=TRACE_DIR,
    )
    LAST_RESULTS = res
    out = np.empty((B, N, DIM), np.float32)
    for c in range(NCORES):
        b, qh, g = c // 4, (c // 2) % 2, c % 2
        out[b, qh * NQ:(qh + 1) * NQ, g * HD:(g + 1) * HD] = \
            res.results[c]["outT"].T
    return out
